# revision 39
# baseline (speedup 1.0000x reference)
"""Trainium2 Bass kernel for nn_Basic_Aggregator (gnn_message_passing).

Math: out[b, i, :] = sum_j node_j[b, j, :]  (sum over node axis, broadcast
back to every row).  edge_ij is unused by the computation.

Sharding: data-parallel over batch B=16 across 8 cores (2 batches/core).
Each core reads its [2, 20000, 64] slab (cast to fp16 host-side; the
2e-2 rel-err gate leaves ~30x margin) and reduces each batch to a [64]
f32 sum on device.  The broadcast back to [20000, 64] is pure
replication, done host-side during unshard.  No cross-core
communication.

Layout: 20000 rows = 125 partitions x 160 rows, so a batch is a fully
contiguous [125, 20480 B] fp16 region per core with no remainder.
"""

import numpy as np

B, SIZE, D = 16, 20000, 64
N_CORES = 8
B_LOCAL = B // N_CORES  # 2
P = 125                 # partitions used; 125 * 160 = 20000 rows
NG = 160                # rows per partition
W = NG * D              # 10240 f32 per partition

_STATE = {}

# Results of the most recent device run (for test harness introspection).
LAST_RESULT = None


def install_axon_ntff_hook_shim():
    """Provide antenv.axon_hooks if the image's antenv lacks it, so
    BASS_TRACE=1 profiling works.  The hook drives NTFF capture via the
    stable C ABI of the injected PJRT plugin .so (same contract the boot
    script uses when the module is present)."""
    import sys as _sys
    import types
    import ctypes
    import contextlib

    if "antenv.axon_hooks" in _sys.modules:
        return
    try:
        import antenv.axon_hooks  # noqa: F401
        return
    except ImportError:
        pass

    mod = types.ModuleType("antenv.axon_hooks")
    _state = {"hook": None}

    def set_axon_ntff_profile_hook(h):
        _state["hook"] = h

    def get_axon_ntff_profile_hook():
        if _state["hook"] is not None:
            return _state["hook"]
        so_path = "/opt/axon/libaxon_pjrt.so"
        try:
            lib = ctypes.CDLL(so_path)
        except OSError:
            return None
        if not hasattr(lib, "axon_start_nrt_profile"):
            return None
        lib.axon_start_nrt_profile.argtypes = [
            ctypes.POINTER(ctypes.c_int64),
            ctypes.c_size_t,
        ]
        lib.axon_start_nrt_profile.restype = ctypes.c_int64
        lib.axon_stop_nrt_profile.argtypes = [ctypes.c_char_p]
        lib.axon_stop_nrt_profile.restype = ctypes.c_int64

        @contextlib.contextmanager
        def _hook(output_dir, device_ids):
            import jax

            jax.devices()
            if device_ids:
                ids = (ctypes.c_int64 * len(device_ids))(*device_ids)
                rc = lib.axon_start_nrt_profile(ids, len(device_ids))
            else:
                rc = lib.axon_start_nrt_profile(None, 0)
            if rc != 0:
                raise RuntimeError(f"axon_start_nrt_profile rc={rc}")
            try:
                yield
            finally:
                n = lib.axon_stop_nrt_profile(str(output_dir).encode())
                if n < 0:
                    raise RuntimeError(f"axon_stop_nrt_profile rc={n}")
                if n == 0:
                    print(
                        f"profile: ZERO FILES written to {output_dir}",
                        file=_sys.stderr,
                    )

        _state["hook"] = _hook
        return _hook

    mod.set_axon_ntff_profile_hook = set_axon_ntff_profile_hook
    mod.get_axon_ntff_profile_hook = get_axon_ntff_profile_hook
    _sys.modules["antenv.axon_hooks"] = mod


def _patch_drain_split():
    """The walrus build in this container accepts at most one sync-wait
    command per instruction; Tile's kernel-tail drain collects one wait per
    dangling proc (6 here) onto a single Drain.  Split it into a chain of
    single-wait drains on the same engine — identical semantics."""
    from concourse import tile
    import concourse.mybir as mybir
    from concourse.vector_clock import ScopedClock

    if getattr(tile.TileContext, "_ant_drain_split", False):
        return

    def _drain_and_barrier(self, tick_clock, wait_clock):
        drain_inst = self.nc.sync.drain()
        wait_clock.add_sem_waits(
            drain_inst.ins, ScopedClock({None: tick_clock.global_clock})
        )
        si = drain_inst.ins.sync_info
        if si is not None and si.on_wait and len(si.on_wait) > 1:
            waits = list(si.on_wait)
            upds = list(si.on_update or [])
            drain_inst.ins.sync_info = mybir.SyncInfo(
                on_wait=[waits[0]], on_update=[]
            )
            for i, w in enumerate(waits[1:]):
                extra = self.nc.sync.drain()
                extra.ins.sync_info = mybir.SyncInfo(
                    on_wait=[w],
                    on_update=upds if i == len(waits) - 2 else [],
                )

        self.nc.all_engine_barrier()
        assert self.sems is not None
        popped = self.nc._tile_sem_poison_stack.pop()
        assert popped is self._sem_poison
        self.nc.clear_and_free_semaphores(list(self.sems.allocated().values()))
        self.nc.all_engine_barrier()

    tile.TileContext._drain_and_barrier = _drain_and_barrier
    tile.TileContext._ant_drain_split = True


def _build_nc():
    import concourse.bass as bass
    import concourse.mybir as mybir
    from concourse import tile

    _patch_drain_split()

    f32 = mybir.dt.float32
    f16 = mybir.dt.float16
    nc = bass.Bass()
    x = nc.declare_dram_parameter("x", [B_LOCAL, SIZE, D], f16, isOutput=False)
    y = nc.declare_dram_parameter("y", [B_LOCAL, D], f32, isOutput=True)

    # Device computes only the per-batch [64] sums.  All loads ride the
    # gpsimd SWDGE queue, which fans descriptors across all 16 SDMA
    # engines (~175-195 GB/s measured; the two HWDGE rings share a single
    # 5-engine bundle capped at ~133 GB/s, and mixing queues makes the
    # shared engines 64-68 straggle).  Total DMA instructions stay <= 8
    # (walrus sem-lane limit).
    with tile.TileContext(nc) as tc:
        with (
            tc.tile_pool(name="io", bufs=1) as io,
            tc.tile_pool(name="small", bufs=1) as small,
            tc.tile_pool(name="psum", bufs=2, space="PSUM") as psum,
        ):
            # ones column [125,1]: matmul partition-reduces part -> [1, 64]
            ones_col = small.tile([P, 1], f32, tag="ones_col")
            nc.vector.memset(ones_col[:], 1.0)

            # Phase 1: all loads up front — pure SWDGE.  b0 in two halves;
            # b1 big-to-small so the final chunk's post-landing fold
            # chain (the serial tail) is short.
            xin = {}
            chunks = {0: [80, 80], 1: [96, 48, 16]}
            # Issue order: b0 first, then b1 big-to-small so only short
            # fold chains stay gated at the end.
            order = [(0, 0), (0, 1), (1, 0), (1, 1), (1, 2)]
            offs = {}
            for b in range(B_LOCAL):
                t = io.tile([P, W], f16, tag=f"in{b}")
                xin[b] = t
                o = 0
                for ci, cg in enumerate(chunks[b]):
                    offs[b, ci] = (o, cg)
                    o += cg
            # All chunks ride SWDGE: routing even the tiny tail chunk via
            # the HWDGE sync ring measured ~6 us WORSE (its packets
            # interleave through the whole load phase on the shared
            # engines 64-68, which are already the SWDGE stragglers).
            for b, ci in order:
                xb = x[b].rearrange("(p w) d -> p (w d)", p=P)  # [125, 10240]
                o, cg = offs[b, ci]
                eng = nc.gpsimd
                eng.dma_start(
                    out=xin[b][:, o * D : (o + cg) * D],
                    in_=xb[:, o * D : (o + cg) * D],
                )

            def fold_to_64(src, rows, tag, eng=None):
                # [125, rows*64] -> [125, 64] fp16 via halving tensor adds;
                # handles odd row counts by folding the odd tail back in.
                # eng selects the engine (default DVE; Pool for chains that
                # should run in parallel with DVE's).
                if eng is None:
                    eng = nc.vector
                s = src
                n = rows * D
                i = 0
                while n > D:
                    half = n // 2
                    if (n // D) % 2 == 1:   # odd rows: peel top row
                        half = (n - D) // 2
                        t2 = small.tile([P, half], f16, tag=f"{tag}{i}")
                        eng.tensor_tensor(
                            t2[:], s[:, :half], s[:, half : 2 * half],
                            op=mybir.AluOpType.add,
                        )
                        eng.tensor_tensor(
                            t2[:, :D], t2[:, :D], s[:, 2 * half : n],
                            op=mybir.AluOpType.add,
                        )
                    else:
                        t2 = small.tile([P, half], f16, tag=f"{tag}{i}")
                        eng.tensor_tensor(
                            t2[:], s[:, :half], s[:, half:n],
                            op=mybir.AluOpType.add,
                        )
                    s, n, i = t2, half, i + 1
                return s

            # Phase 2: fold chunks in load-issue order as their DMAs
            # complete, combine in f32, PE partition-reduce into one
            # shared [1, 128] PSUM tile, one copy to SBUF, one tiny store.
            stage = small.tile([1, B_LOCAL * D], f32, tag="stage")
            tot = psum.tile([1, B_LOCAL * D], f32, tag="tot")

            def chunk_view(b, ci):
                o, cg = offs[b, ci]
                return xin[b][:, o * D : (o + cg) * D], cg

            parts16 = {}
            for b, ci in order:
                src, cg = chunk_view(b, ci)
                # (Offloading a chain to Pool via gpsimd.tensor_tensor
                # fails to compile without a Q7 library load; all chains
                # stay on DVE.)
                parts16[b, ci] = fold_to_64(src, cg, f"r{b}{ci}")
                if all((b2, c2) in parts16
                       for b2 in (b,) for c2 in range(len(chunks[b]))):
                    ps = [parts16[b, c2] for c2 in range(len(chunks[b]))]
                    part = small.tile([P, D], f32, tag=f"part{b}")
                    nc.vector.tensor_tensor(
                        part[:], ps[0][:], ps[1][:], op=mybir.AluOpType.add
                    )
                    for extra in ps[2:]:
                        nc.vector.tensor_tensor(
                            part[:], part[:], extra[:], op=mybir.AluOpType.add
                        )
                    nc.tensor.matmul(
                        tot[:, b * D : (b + 1) * D], ones_col[:], part[:],
                        start=True, stop=True,
                    )
            nc.vector.tensor_copy(stage[:], tot[:])

            nc.sync.dma_start(
                out=y.rearrange("b d -> (b d)").unsqueeze(0), in_=stage[:]
            )

    return nc


def _build_nc_raw():
    """Raw-bass build (no TileContext): same dataflow as _build_nc but
    with hand-placed semaphores.  Saves ~5 us of Tile pool/barrier
    preamble and teardown (first DMA dispatches at ~5 us instead of
    ~9.5 us; single-instruction teardown instead of drain chains)."""
    import contextlib
    import concourse.bass as bass
    import concourse.mybir as mybir

    f32 = mybir.dt.float32
    f16 = mybir.dt.float16
    add = mybir.AluOpType.add
    nc = bass.Bass()
    x = nc.declare_dram_parameter("x", [B_LOCAL, SIZE, D], f16, isOutput=False)
    y = nc.declare_dram_parameter("y", [B_LOCAL, D], f32, isOutput=True)

    chunks = {0: [80, 80], 1: [96, 48, 16]}
    order = [(0, 0), (0, 1), (1, 0), (1, 1), (1, 2)]

    with contextlib.ExitStack() as ctx:
        def sb(name, shape, dtype):
            return ctx.enter_context(nc.sbuf_tensor(name, shape, dtype))

        def sem(name):
            return ctx.enter_context(nc.semaphore(name))

        xin = {b: sb(f"xin{b}", [P, W], f16) for b in range(B_LOCAL)}
        ones = sb("ones", [P, 1], f32)
        stage = sb("stage", [1, B_LOCAL * D], f32)
        tots = {
            b: ctx.enter_context(nc.psum_tensor(f"tot{b}", [1, D], f32))
            for b in range(B_LOCAL)
        }

        offs = {}
        for b in range(B_LOCAL):
            o = 0
            for ci, cg in enumerate(chunks[b]):
                offs[b, ci] = (o, cg)
                o += cg

        load_sems = {}
        for b, ci in order:
            xb = x[b].rearrange("(p w) d -> p (w d)", p=P)
            o, cg = offs[b, ci]
            s = sem(f"sl{b}{ci}")
            load_sems[b, ci] = s
            nc.gpsimd.dma_start(
                out=xin[b][:, o * D : (o + cg) * D],
                in_=xb[:, o * D : (o + cg) * D],
            ).then_inc(s, 16)

        s_dve = sem("s_dve")
        s_pe = sem("s_pe")
        s_copy = sem("s_copy")
        s_store = sem("s_store")

        nc.vector.memset(ones[:], 1.0)

        cnt = [0]

        def scratch(n_elem, dtype):
            cnt[0] += 1
            return sb(f"sc{cnt[0]}", [P, n_elem], dtype)

        def fold_to_64(src, rows):
            s = src
            n = rows * D
            while n > D:
                half = n // 2
                if (n // D) % 2 == 1:   # odd rows: peel top row
                    half = (n - D) // 2
                    t2 = scratch(half, f16)
                    nc.vector.tensor_tensor(
                        t2[:], s[:, :half], s[:, half : 2 * half], op=add
                    )
                    nc.vector.tensor_tensor(
                        t2[:, :D], t2[:, :D], s[:, 2 * half : n], op=add
                    )
                else:
                    t2 = scratch(half, f16)
                    nc.vector.tensor_tensor(t2[:], s[:, :half], s[:, half:n], op=add)
                s, n = t2, half
            return s

        for b in range(B_LOCAL):
            parts16 = []
            for ci, cg in enumerate(chunks[b]):
                o, _ = offs[b, ci]
                nc.vector.wait_ge(load_sems[b, ci], 16)
                parts16.append(fold_to_64(xin[b][:, o * D : (o + cg) * D], cg))
            part = scratch(D, f32)
            last = nc.vector.tensor_tensor(
                part[:], parts16[0][:], parts16[1][:], op=add
            )
            for extra in parts16[2:]:
                last = nc.vector.tensor_tensor(part[:], part[:], extra[:], op=add)
            last.then_inc(s_dve, 1)

            nc.tensor.wait_ge(s_dve, b + 1)
            nc.tensor.matmul(
                tots[b][:], ones[:], part[:], start=True, stop=True,
            ).then_inc(s_pe, 1)

            nc.vector.wait_ge(s_pe, b + 1)
            nc.vector.tensor_copy(
                stage[:, b * D : (b + 1) * D], tots[b][:]
            ).then_inc(s_copy, 1)

        nc.sync.wait_ge(s_copy, B_LOCAL)
        nc.sync.dma_start(
            out=y.rearrange("b d -> (b d)").unsqueeze(0), in_=stage[:]
        ).then_inc(s_store, 16)
        nc.sync.wait_ge(s_store, 16)
        # Reset sems so a re-execution of the loaded NEFF starts clean.
        for s in [*load_sems.values(), s_dve, s_pe, s_copy, s_store]:
            nc.sync.sem_clear(s)

    return nc


def _get_nc():
    # _build_nc_raw (above) trims ~1 us of Tile preamble but its plain
    # .then_inc SWDGE completion sems release at descriptor emission, not
    # data landing -> races.  Use the proven TileContext build.
    if "nc" not in _STATE:
        _STATE["nc"] = _build_nc()
    return _STATE["nc"]


def kernel(node_j, edge_ij=None):
    global LAST_RESULT
    install_axon_ntff_hook_shim()
    from concourse.bass_utils import run_bass_kernel_spmd

    node_j = np.asarray(node_j)
    assert node_j.shape == (B, SIZE, D), node_j.shape
    x16 = np.ascontiguousarray(node_j, dtype=np.float16)

    nc = _get_nc()
    in_maps = [
        {"x": x16[i * B_LOCAL:(i + 1) * B_LOCAL]} for i in range(N_CORES)
    ]
    res = run_bass_kernel_spmd(nc, in_maps, core_ids=list(range(N_CORES)))
    LAST_RESULT = res
    sums = np.concatenate([r["y"] for r in res.results], axis=0)  # [16, 64]
    out = np.empty((B, SIZE, D), dtype=np.float32)
    np.copyto(out, sums[:, None, :])
    return out



# revision 40
# speedup vs baseline: 1.0340x; 1.0340x over previous
"""Trainium2 Bass kernel for nn_Basic_Aggregator (gnn_message_passing).

Math: out[b, i, :] = sum_j node_j[b, j, :]  (sum over node axis, broadcast
back to every row).  edge_ij is unused by the computation.

Sharding: data-parallel over batch B=16 across 8 cores (2 batches/core).
Each core reads its [2, 20000, 64] slab (cast to fp16 host-side; the
2e-2 rel-err gate leaves ~30x margin) and reduces each batch to a [64]
f32 sum on device.  The broadcast back to [20000, 64] is pure
replication, done host-side during unshard.  No cross-core
communication.

Layout: 20000 rows = 125 partitions x 160 rows, so a batch is a fully
contiguous [125, 20480 B] fp16 region per core with no remainder.
"""

import numpy as np

B, SIZE, D = 16, 20000, 64
N_CORES = 8
B_LOCAL = B // N_CORES  # 2
P = 125                 # partitions used; 125 * 160 = 20000 rows
NG = 160                # rows per partition
W = NG * D              # 10240 f32 per partition

_STATE = {}

# Results of the most recent device run (for test harness introspection).
LAST_RESULT = None


def install_axon_ntff_hook_shim():
    """Provide antenv.axon_hooks if the image's antenv lacks it, so
    BASS_TRACE=1 profiling works.  The hook drives NTFF capture via the
    stable C ABI of the injected PJRT plugin .so (same contract the boot
    script uses when the module is present)."""
    import sys as _sys
    import types
    import ctypes
    import contextlib

    if "antenv.axon_hooks" in _sys.modules:
        return
    try:
        import antenv.axon_hooks  # noqa: F401
        return
    except ImportError:
        pass

    mod = types.ModuleType("antenv.axon_hooks")
    _state = {"hook": None}

    def set_axon_ntff_profile_hook(h):
        _state["hook"] = h

    def get_axon_ntff_profile_hook():
        if _state["hook"] is not None:
            return _state["hook"]
        so_path = "/opt/axon/libaxon_pjrt.so"
        try:
            lib = ctypes.CDLL(so_path)
        except OSError:
            return None
        if not hasattr(lib, "axon_start_nrt_profile"):
            return None
        lib.axon_start_nrt_profile.argtypes = [
            ctypes.POINTER(ctypes.c_int64),
            ctypes.c_size_t,
        ]
        lib.axon_start_nrt_profile.restype = ctypes.c_int64
        lib.axon_stop_nrt_profile.argtypes = [ctypes.c_char_p]
        lib.axon_stop_nrt_profile.restype = ctypes.c_int64

        @contextlib.contextmanager
        def _hook(output_dir, device_ids):
            import jax

            jax.devices()
            if device_ids:
                ids = (ctypes.c_int64 * len(device_ids))(*device_ids)
                rc = lib.axon_start_nrt_profile(ids, len(device_ids))
            else:
                rc = lib.axon_start_nrt_profile(None, 0)
            if rc != 0:
                raise RuntimeError(f"axon_start_nrt_profile rc={rc}")
            try:
                yield
            finally:
                n = lib.axon_stop_nrt_profile(str(output_dir).encode())
                if n < 0:
                    raise RuntimeError(f"axon_stop_nrt_profile rc={n}")
                if n == 0:
                    print(
                        f"profile: ZERO FILES written to {output_dir}",
                        file=_sys.stderr,
                    )

        _state["hook"] = _hook
        return _hook

    mod.set_axon_ntff_profile_hook = set_axon_ntff_profile_hook
    mod.get_axon_ntff_profile_hook = get_axon_ntff_profile_hook
    _sys.modules["antenv.axon_hooks"] = mod


def _patch_drain_split():
    """The walrus build in this container accepts at most one sync-wait
    command per instruction; Tile's kernel-tail drain collects one wait per
    dangling proc (6 here) onto a single Drain.  Split it into a chain of
    single-wait drains on the same engine — identical semantics."""
    from concourse import tile
    import concourse.mybir as mybir
    from concourse.vector_clock import ScopedClock

    if getattr(tile.TileContext, "_ant_drain_split", False):
        return

    def _drain_and_barrier(self, tick_clock, wait_clock):
        drain_inst = self.nc.sync.drain()
        wait_clock.add_sem_waits(
            drain_inst.ins, ScopedClock({None: tick_clock.global_clock})
        )
        si = drain_inst.ins.sync_info
        if si is not None and si.on_wait and len(si.on_wait) > 1:
            waits = list(si.on_wait)
            upds = list(si.on_update or [])
            drain_inst.ins.sync_info = mybir.SyncInfo(
                on_wait=[waits[0]], on_update=[]
            )
            for i, w in enumerate(waits[1:]):
                extra = self.nc.sync.drain()
                extra.ins.sync_info = mybir.SyncInfo(
                    on_wait=[w],
                    on_update=upds if i == len(waits) - 2 else [],
                )

        self.nc.all_engine_barrier()
        assert self.sems is not None
        popped = self.nc._tile_sem_poison_stack.pop()
        assert popped is self._sem_poison
        self.nc.clear_and_free_semaphores(list(self.sems.allocated().values()))
        # No trailing all_engine_barrier: each engine's sem clears precede
        # its halt in program order, and the next execution begins with its
        # own entry barrier, so cross-engine sync here only costs time.

    tile.TileContext._drain_and_barrier = _drain_and_barrier
    tile.TileContext._ant_drain_split = True


def _build_nc():
    import concourse.bass as bass
    import concourse.mybir as mybir
    from concourse import tile

    _patch_drain_split()

    f32 = mybir.dt.float32
    f16 = mybir.dt.float16
    nc = bass.Bass()
    x = nc.declare_dram_parameter("x", [B_LOCAL, SIZE, D], f16, isOutput=False)
    y = nc.declare_dram_parameter("y", [B_LOCAL, D], f32, isOutput=True)

    # Device computes only the per-batch [64] sums.  All loads ride the
    # gpsimd SWDGE queue, which fans descriptors across all 16 SDMA
    # engines (~175-195 GB/s measured; the two HWDGE rings share a single
    # 5-engine bundle capped at ~133 GB/s, and mixing queues makes the
    # shared engines 64-68 straggle).  Total DMA instructions stay <= 8
    # (walrus sem-lane limit).
    with tile.TileContext(nc) as tc:
        with (
            tc.tile_pool(name="io", bufs=1) as io,
            tc.tile_pool(name="small", bufs=1) as small,
            tc.tile_pool(name="psum", bufs=2, space="PSUM") as psum,
        ):
            # ones column [125,1]: matmul partition-reduces part -> [1, 64]
            ones_col = small.tile([P, 1], f32, tag="ones_col")
            nc.vector.memset(ones_col[:], 1.0)

            # Phase 1: all loads up front — pure SWDGE.  b0 in two halves;
            # b1 big-to-small so the final chunk's post-landing fold
            # chain (the serial tail) is short.
            xin = {}
            chunks = {0: [80, 80], 1: [96, 48, 16]}
            # Issue order: b0 first, then b1 big-to-small so only short
            # fold chains stay gated at the end.
            order = [(0, 0), (0, 1), (1, 0), (1, 1), (1, 2)]
            offs = {}
            for b in range(B_LOCAL):
                t = io.tile([P, W], f16, tag=f"in{b}")
                xin[b] = t
                o = 0
                for ci, cg in enumerate(chunks[b]):
                    offs[b, ci] = (o, cg)
                    o += cg
            # All chunks ride SWDGE: routing even the tiny tail chunk via
            # the HWDGE sync ring measured ~6 us WORSE (its packets
            # interleave through the whole load phase on the shared
            # engines 64-68, which are already the SWDGE stragglers).
            for b, ci in order:
                xb = x[b].rearrange("(p w) d -> p (w d)", p=P)  # [125, 10240]
                o, cg = offs[b, ci]
                eng = nc.gpsimd
                eng.dma_start(
                    out=xin[b][:, o * D : (o + cg) * D],
                    in_=xb[:, o * D : (o + cg) * D],
                )

            def fold_to_64(src, rows, tag, eng=None):
                # [125, rows*64] -> [125, 64] fp16 via halving tensor adds;
                # handles odd row counts by folding the odd tail back in.
                # eng selects the engine (default DVE; Pool for chains that
                # should run in parallel with DVE's).
                if eng is None:
                    eng = nc.vector
                s = src
                n = rows * D
                i = 0
                while n > D:
                    half = n // 2
                    if (n // D) % 2 == 1:   # odd rows: peel top row
                        half = (n - D) // 2
                        t2 = small.tile([P, half], f16, tag=f"{tag}{i}")
                        eng.tensor_tensor(
                            t2[:], s[:, :half], s[:, half : 2 * half],
                            op=mybir.AluOpType.add,
                        )
                        eng.tensor_tensor(
                            t2[:, :D], t2[:, :D], s[:, 2 * half : n],
                            op=mybir.AluOpType.add,
                        )
                    else:
                        t2 = small.tile([P, half], f16, tag=f"{tag}{i}")
                        eng.tensor_tensor(
                            t2[:], s[:, :half], s[:, half:n],
                            op=mybir.AluOpType.add,
                        )
                    s, n, i = t2, half, i + 1
                return s

            # Phase 2: fold chunks in load-issue order as their DMAs
            # complete, combine in f32, PE partition-reduce into one
            # shared [1, 128] PSUM tile, one copy to SBUF, one tiny store.
            stage = small.tile([1, B_LOCAL * D], f32, tag="stage")
            tot = psum.tile([1, B_LOCAL * D], f32, tag="tot")

            def chunk_view(b, ci):
                o, cg = offs[b, ci]
                return xin[b][:, o * D : (o + cg) * D], cg

            parts16 = {}
            for b, ci in order:
                src, cg = chunk_view(b, ci)
                # (Offloading a chain to Pool via gpsimd.tensor_tensor
                # fails to compile without a Q7 library load; all chains
                # stay on DVE.)
                parts16[b, ci] = fold_to_64(src, cg, f"r{b}{ci}")
                if all((b2, c2) in parts16
                       for b2 in (b,) for c2 in range(len(chunks[b]))):
                    ps = [parts16[b, c2] for c2 in range(len(chunks[b]))]
                    part = small.tile([P, D], f32, tag=f"part{b}")
                    nc.vector.tensor_tensor(
                        part[:], ps[0][:], ps[1][:], op=mybir.AluOpType.add
                    )
                    for extra in ps[2:]:
                        nc.vector.tensor_tensor(
                            part[:], part[:], extra[:], op=mybir.AluOpType.add
                        )
                    nc.tensor.matmul(
                        tot[:, b * D : (b + 1) * D], ones_col[:], part[:],
                        start=True, stop=True,
                    )
            nc.vector.tensor_copy(stage[:], tot[:])

            nc.sync.dma_start(
                out=y.rearrange("b d -> (b d)").unsqueeze(0), in_=stage[:]
            )

    return nc


def _build_nc_raw():
    """Raw-bass build (no TileContext): same dataflow as _build_nc but
    with hand-placed semaphores.  Saves ~5 us of Tile pool/barrier
    preamble and teardown (first DMA dispatches at ~5 us instead of
    ~9.5 us; single-instruction teardown instead of drain chains)."""
    import contextlib
    import concourse.bass as bass
    import concourse.mybir as mybir

    f32 = mybir.dt.float32
    f16 = mybir.dt.float16
    add = mybir.AluOpType.add
    nc = bass.Bass()
    x = nc.declare_dram_parameter("x", [B_LOCAL, SIZE, D], f16, isOutput=False)
    y = nc.declare_dram_parameter("y", [B_LOCAL, D], f32, isOutput=True)

    chunks = {0: [80, 80], 1: [96, 48, 16]}
    order = [(0, 0), (0, 1), (1, 0), (1, 1), (1, 2)]

    with contextlib.ExitStack() as ctx:
        def sb(name, shape, dtype):
            return ctx.enter_context(nc.sbuf_tensor(name, shape, dtype))

        def sem(name):
            return ctx.enter_context(nc.semaphore(name))

        xin = {b: sb(f"xin{b}", [P, W], f16) for b in range(B_LOCAL)}
        ones = sb("ones", [P, 1], f32)
        stage = sb("stage", [1, B_LOCAL * D], f32)
        tots = {
            b: ctx.enter_context(nc.psum_tensor(f"tot{b}", [1, D], f32))
            for b in range(B_LOCAL)
        }

        offs = {}
        for b in range(B_LOCAL):
            o = 0
            for ci, cg in enumerate(chunks[b]):
                offs[b, ci] = (o, cg)
                o += cg

        load_sems = {}
        for b, ci in order:
            xb = x[b].rearrange("(p w) d -> p (w d)", p=P)
            o, cg = offs[b, ci]
            s = sem(f"sl{b}{ci}")
            load_sems[b, ci] = s
            nc.gpsimd.dma_start(
                out=xin[b][:, o * D : (o + cg) * D],
                in_=xb[:, o * D : (o + cg) * D],
            ).then_inc(s, 16)

        s_dve = sem("s_dve")
        s_pe = sem("s_pe")
        s_copy = sem("s_copy")
        s_store = sem("s_store")

        nc.vector.memset(ones[:], 1.0)

        cnt = [0]

        def scratch(n_elem, dtype):
            cnt[0] += 1
            return sb(f"sc{cnt[0]}", [P, n_elem], dtype)

        def fold_to_64(src, rows):
            s = src
            n = rows * D
            while n > D:
                half = n // 2
                if (n // D) % 2 == 1:   # odd rows: peel top row
                    half = (n - D) // 2
                    t2 = scratch(half, f16)
                    nc.vector.tensor_tensor(
                        t2[:], s[:, :half], s[:, half : 2 * half], op=add
                    )
                    nc.vector.tensor_tensor(
                        t2[:, :D], t2[:, :D], s[:, 2 * half : n], op=add
                    )
                else:
                    t2 = scratch(half, f16)
                    nc.vector.tensor_tensor(t2[:], s[:, :half], s[:, half:n], op=add)
                s, n = t2, half
            return s

        for b in range(B_LOCAL):
            parts16 = []
            for ci, cg in enumerate(chunks[b]):
                o, _ = offs[b, ci]
                nc.vector.wait_ge(load_sems[b, ci], 16)
                parts16.append(fold_to_64(xin[b][:, o * D : (o + cg) * D], cg))
            part = scratch(D, f32)
            last = nc.vector.tensor_tensor(
                part[:], parts16[0][:], parts16[1][:], op=add
            )
            for extra in parts16[2:]:
                last = nc.vector.tensor_tensor(part[:], part[:], extra[:], op=add)
            last.then_inc(s_dve, 1)

            nc.tensor.wait_ge(s_dve, b + 1)
            nc.tensor.matmul(
                tots[b][:], ones[:], part[:], start=True, stop=True,
            ).then_inc(s_pe, 1)

            nc.vector.wait_ge(s_pe, b + 1)
            nc.vector.tensor_copy(
                stage[:, b * D : (b + 1) * D], tots[b][:]
            ).then_inc(s_copy, 1)

        nc.sync.wait_ge(s_copy, B_LOCAL)
        nc.sync.dma_start(
            out=y.rearrange("b d -> (b d)").unsqueeze(0), in_=stage[:]
        ).then_inc(s_store, 16)
        nc.sync.wait_ge(s_store, 16)
        # Reset sems so a re-execution of the loaded NEFF starts clean.
        for s in [*load_sems.values(), s_dve, s_pe, s_copy, s_store]:
            nc.sync.sem_clear(s)

    return nc


def _get_nc():
    # _build_nc_raw (above) trims ~1 us of Tile preamble but its plain
    # .then_inc SWDGE completion sems release at descriptor emission, not
    # data landing -> races.  Use the proven TileContext build.
    if "nc" not in _STATE:
        _STATE["nc"] = _build_nc()
    return _STATE["nc"]


def kernel(node_j, edge_ij=None):
    global LAST_RESULT
    install_axon_ntff_hook_shim()
    from concourse.bass_utils import run_bass_kernel_spmd

    node_j = np.asarray(node_j)
    assert node_j.shape == (B, SIZE, D), node_j.shape
    x16 = np.ascontiguousarray(node_j, dtype=np.float16)

    nc = _get_nc()
    in_maps = [
        {"x": x16[i * B_LOCAL:(i + 1) * B_LOCAL]} for i in range(N_CORES)
    ]
    res = run_bass_kernel_spmd(nc, in_maps, core_ids=list(range(N_CORES)))
    LAST_RESULT = res
    sums = np.concatenate([r["y"] for r in res.results], axis=0)  # [16, 64]
    out = np.empty((B, SIZE, D), dtype=np.float32)
    np.copyto(out, sums[:, None, :])
    return out



# revision 42
# speedup vs baseline: 1.0461x; 1.0118x over previous
"""Trainium2 Bass kernel for nn_Basic_Aggregator (gnn_message_passing).

Math: out[b, i, :] = sum_j node_j[b, j, :]  (sum over node axis, broadcast
back to every row).  edge_ij is unused by the computation.

Sharding: data-parallel over batch B=16 across 8 cores (2 batches/core).
Each core reads its [2, 20000, 64] slab (cast to fp16 host-side; the
2e-2 rel-err gate leaves ~30x margin) and reduces each batch to a [64]
f32 sum on device.  The broadcast back to [20000, 64] is pure
replication, done host-side during unshard.  No cross-core
communication.

Layout: 20000 rows = 125 partitions x 160 rows, so a batch is a fully
contiguous [125, 20480 B] fp16 region per core with no remainder.
"""

import numpy as np

B, SIZE, D = 16, 20000, 64
N_CORES = 8
B_LOCAL = B // N_CORES  # 2
P = 125                 # partitions used; 125 * 160 = 20000 rows
NG = 160                # rows per partition
W = NG * D              # 10240 f32 per partition

_STATE = {}

# Results of the most recent device run (for test harness introspection).
LAST_RESULT = None


def install_axon_ntff_hook_shim():
    """Provide antenv.axon_hooks if the image's antenv lacks it, so
    BASS_TRACE=1 profiling works.  The hook drives NTFF capture via the
    stable C ABI of the injected PJRT plugin .so (same contract the boot
    script uses when the module is present)."""
    import sys as _sys
    import types
    import ctypes
    import contextlib

    if "antenv.axon_hooks" in _sys.modules:
        return
    try:
        import antenv.axon_hooks  # noqa: F401
        return
    except ImportError:
        pass

    mod = types.ModuleType("antenv.axon_hooks")
    _state = {"hook": None}

    def set_axon_ntff_profile_hook(h):
        _state["hook"] = h

    def get_axon_ntff_profile_hook():
        if _state["hook"] is not None:
            return _state["hook"]
        so_path = "/opt/axon/libaxon_pjrt.so"
        try:
            lib = ctypes.CDLL(so_path)
        except OSError:
            return None
        if not hasattr(lib, "axon_start_nrt_profile"):
            return None
        lib.axon_start_nrt_profile.argtypes = [
            ctypes.POINTER(ctypes.c_int64),
            ctypes.c_size_t,
        ]
        lib.axon_start_nrt_profile.restype = ctypes.c_int64
        lib.axon_stop_nrt_profile.argtypes = [ctypes.c_char_p]
        lib.axon_stop_nrt_profile.restype = ctypes.c_int64

        @contextlib.contextmanager
        def _hook(output_dir, device_ids):
            import jax

            jax.devices()
            if device_ids:
                ids = (ctypes.c_int64 * len(device_ids))(*device_ids)
                rc = lib.axon_start_nrt_profile(ids, len(device_ids))
            else:
                rc = lib.axon_start_nrt_profile(None, 0)
            if rc != 0:
                raise RuntimeError(f"axon_start_nrt_profile rc={rc}")
            try:
                yield
            finally:
                n = lib.axon_stop_nrt_profile(str(output_dir).encode())
                if n < 0:
                    raise RuntimeError(f"axon_stop_nrt_profile rc={n}")
                if n == 0:
                    print(
                        f"profile: ZERO FILES written to {output_dir}",
                        file=_sys.stderr,
                    )

        _state["hook"] = _hook
        return _hook

    mod.set_axon_ntff_profile_hook = set_axon_ntff_profile_hook
    mod.get_axon_ntff_profile_hook = get_axon_ntff_profile_hook
    _sys.modules["antenv.axon_hooks"] = mod


def _patch_drain_split():
    """The walrus build in this container accepts at most one sync-wait
    command per instruction; Tile's kernel-tail drain collects one wait per
    dangling proc (6 here) onto a single Drain.  Split it into a chain of
    single-wait drains on the same engine — identical semantics."""
    from concourse import tile
    import concourse.mybir as mybir
    from concourse.vector_clock import ScopedClock

    if getattr(tile.TileContext, "_ant_drain_split", False):
        return

    def _drain_and_barrier(self, tick_clock, wait_clock):
        drain_inst = self.nc.sync.drain()
        wait_clock.add_sem_waits(
            drain_inst.ins, ScopedClock({None: tick_clock.global_clock})
        )
        si = drain_inst.ins.sync_info
        if si is not None and si.on_wait and len(si.on_wait) > 1:
            waits = list(si.on_wait)
            upds = list(si.on_update or [])
            drain_inst.ins.sync_info = mybir.SyncInfo(
                on_wait=[waits[0]], on_update=[]
            )
            for i, w in enumerate(waits[1:]):
                extra = self.nc.sync.drain()
                extra.ins.sync_info = mybir.SyncInfo(
                    on_wait=[w],
                    on_update=upds if i == len(waits) - 2 else [],
                )

        self.nc.all_engine_barrier()
        assert self.sems is not None
        popped = self.nc._tile_sem_poison_stack.pop()
        assert popped is self._sem_poison
        self.nc.clear_and_free_semaphores(list(self.sems.allocated().values()))
        # No trailing all_engine_barrier (the first one is required for a
        # clean build): each engine's sem clears precede its halt in
        # program order, and the next execution begins with its own entry
        # barrier, so the trailing cross-engine sync only costs time.

    tile.TileContext._drain_and_barrier = _drain_and_barrier
    tile.TileContext._ant_drain_split = True


def _build_nc():
    import concourse.bass as bass
    import concourse.mybir as mybir
    from concourse import tile

    _patch_drain_split()

    f32 = mybir.dt.float32
    f16 = mybir.dt.float16
    nc = bass.Bass()
    x = nc.declare_dram_parameter("x", [B_LOCAL, SIZE, D], f16, isOutput=False)
    y = nc.declare_dram_parameter("y", [B_LOCAL, D], f32, isOutput=True)

    # Device computes only the per-batch [64] sums.  All loads ride the
    # gpsimd SWDGE queue, which fans descriptors across all 16 SDMA
    # engines (~175-195 GB/s measured; the two HWDGE rings share a single
    # 5-engine bundle capped at ~133 GB/s, and mixing queues makes the
    # shared engines 64-68 straggle).  Total DMA instructions stay <= 8
    # (walrus sem-lane limit).
    with tile.TileContext(nc) as tc:
        with (
            tc.tile_pool(name="io", bufs=1) as io,
            tc.tile_pool(name="small", bufs=1) as small,
            tc.tile_pool(name="psum", bufs=2, space="PSUM") as psum,
        ):
            # ones column [125,1]: matmul partition-reduces part -> [1, 64]
            ones_col = small.tile([P, 1], f32, tag="ones_col")
            nc.vector.memset(ones_col[:], 1.0)

            # Phase 1: all loads up front — pure SWDGE.  b0 in two halves;
            # b1 big-to-small so the final chunk's post-landing fold
            # chain (the serial tail) is short.
            xin = {}
            chunks = {0: [80, 80], 1: [96, 48, 16]}
            # Issue order: b0 first, then b1 big-to-small so only short
            # fold chains stay gated at the end.
            order = [(0, 0), (0, 1), (1, 0), (1, 1), (1, 2)]
            offs = {}
            for b in range(B_LOCAL):
                t = io.tile([P, W], f16, tag=f"in{b}")
                xin[b] = t
                o = 0
                for ci, cg in enumerate(chunks[b]):
                    offs[b, ci] = (o, cg)
                    o += cg
            # All chunks ride SWDGE: routing even the tiny tail chunk via
            # the HWDGE sync ring measured ~6 us WORSE (its packets
            # interleave through the whole load phase on the shared
            # engines 64-68, which are already the SWDGE stragglers).
            for b, ci in order:
                xb = x[b].rearrange("(p w) d -> p (w d)", p=P)  # [125, 10240]
                o, cg = offs[b, ci]
                eng = nc.gpsimd
                eng.dma_start(
                    out=xin[b][:, o * D : (o + cg) * D],
                    in_=xb[:, o * D : (o + cg) * D],
                )

            def fold_to_64(src, rows, tag, eng=None):
                # [125, rows*64] -> [125, 64] fp16 via halving tensor adds;
                # handles odd row counts by folding the odd tail back in.
                # eng selects the engine (default DVE; Pool for chains that
                # should run in parallel with DVE's).
                if eng is None:
                    eng = nc.vector
                s = src
                n = rows * D
                i = 0
                while n > D:
                    half = n // 2
                    if (n // D) % 2 == 1:   # odd rows: peel top row
                        half = (n - D) // 2
                        t2 = small.tile([P, half], f16, tag=f"{tag}{i}")
                        eng.tensor_tensor(
                            t2[:], s[:, :half], s[:, half : 2 * half],
                            op=mybir.AluOpType.add,
                        )
                        eng.tensor_tensor(
                            t2[:, :D], t2[:, :D], s[:, 2 * half : n],
                            op=mybir.AluOpType.add,
                        )
                    else:
                        t2 = small.tile([P, half], f16, tag=f"{tag}{i}")
                        eng.tensor_tensor(
                            t2[:], s[:, :half], s[:, half:n],
                            op=mybir.AluOpType.add,
                        )
                    s, n, i = t2, half, i + 1
                return s

            # Phase 2: fold chunks in load-issue order as their DMAs
            # complete, combine in f32, PE partition-reduce into one
            # shared [1, 128] PSUM tile, one copy to SBUF, one tiny store.
            stage = small.tile([1, B_LOCAL * D], f32, tag="stage")
            tot = psum.tile([1, B_LOCAL * D], f32, tag="tot")

            def chunk_view(b, ci):
                o, cg = offs[b, ci]
                return xin[b][:, o * D : (o + cg) * D], cg

            parts16 = {}
            for b, ci in order:
                src, cg = chunk_view(b, ci)
                # (Offloading a chain to Pool via gpsimd.tensor_tensor
                # fails to compile without a Q7 library load; all chains
                # stay on DVE.)
                parts16[b, ci] = fold_to_64(src, cg, f"r{b}{ci}")
                if all((b2, c2) in parts16
                       for b2 in (b,) for c2 in range(len(chunks[b]))):
                    ps = [parts16[b, c2] for c2 in range(len(chunks[b]))]
                    part = small.tile([P, D], f32, tag=f"part{b}")
                    nc.vector.tensor_tensor(
                        part[:], ps[0][:], ps[1][:], op=mybir.AluOpType.add
                    )
                    for extra in ps[2:]:
                        nc.vector.tensor_tensor(
                            part[:], part[:], extra[:], op=mybir.AluOpType.add
                        )
                    nc.tensor.matmul(
                        tot[:, b * D : (b + 1) * D], ones_col[:], part[:],
                        start=True, stop=True,
                    )
            nc.vector.tensor_copy(stage[:], tot[:])

            nc.sync.dma_start(
                out=y.rearrange("b d -> (b d)").unsqueeze(0), in_=stage[:]
            )

    return nc


def _build_nc_raw():
    """Raw-bass build (no TileContext): same dataflow as _build_nc but
    with hand-placed semaphores.  Saves ~5 us of Tile pool/barrier
    preamble and teardown (first DMA dispatches at ~5 us instead of
    ~9.5 us; single-instruction teardown instead of drain chains)."""
    import contextlib
    import concourse.bass as bass
    import concourse.mybir as mybir

    f32 = mybir.dt.float32
    f16 = mybir.dt.float16
    add = mybir.AluOpType.add
    nc = bass.Bass()
    x = nc.declare_dram_parameter("x", [B_LOCAL, SIZE, D], f16, isOutput=False)
    y = nc.declare_dram_parameter("y", [B_LOCAL, D], f32, isOutput=True)

    chunks = {0: [80, 80], 1: [96, 48, 16]}
    order = [(0, 0), (0, 1), (1, 0), (1, 1), (1, 2)]

    with contextlib.ExitStack() as ctx:
        def sb(name, shape, dtype):
            return ctx.enter_context(nc.sbuf_tensor(name, shape, dtype))

        def sem(name):
            return ctx.enter_context(nc.semaphore(name))

        xin = {b: sb(f"xin{b}", [P, W], f16) for b in range(B_LOCAL)}
        ones = sb("ones", [P, 1], f32)
        stage = sb("stage", [1, B_LOCAL * D], f32)
        tots = {
            b: ctx.enter_context(nc.psum_tensor(f"tot{b}", [1, D], f32))
            for b in range(B_LOCAL)
        }

        offs = {}
        for b in range(B_LOCAL):
            o = 0
            for ci, cg in enumerate(chunks[b]):
                offs[b, ci] = (o, cg)
                o += cg

        load_sems = {}
        for b, ci in order:
            xb = x[b].rearrange("(p w) d -> p (w d)", p=P)
            o, cg = offs[b, ci]
            s = sem(f"sl{b}{ci}")
            load_sems[b, ci] = s
            nc.gpsimd.dma_start(
                out=xin[b][:, o * D : (o + cg) * D],
                in_=xb[:, o * D : (o + cg) * D],
            ).then_inc(s, 16)

        s_dve = sem("s_dve")
        s_pe = sem("s_pe")
        s_copy = sem("s_copy")
        s_store = sem("s_store")

        nc.vector.memset(ones[:], 1.0)

        cnt = [0]

        def scratch(n_elem, dtype):
            cnt[0] += 1
            return sb(f"sc{cnt[0]}", [P, n_elem], dtype)

        def fold_to_64(src, rows):
            s = src
            n = rows * D
            while n > D:
                half = n // 2
                if (n // D) % 2 == 1:   # odd rows: peel top row
                    half = (n - D) // 2
                    t2 = scratch(half, f16)
                    nc.vector.tensor_tensor(
                        t2[:], s[:, :half], s[:, half : 2 * half], op=add
                    )
                    nc.vector.tensor_tensor(
                        t2[:, :D], t2[:, :D], s[:, 2 * half : n], op=add
                    )
                else:
                    t2 = scratch(half, f16)
                    nc.vector.tensor_tensor(t2[:], s[:, :half], s[:, half:n], op=add)
                s, n = t2, half
            return s

        for b in range(B_LOCAL):
            parts16 = []
            for ci, cg in enumerate(chunks[b]):
                o, _ = offs[b, ci]
                nc.vector.wait_ge(load_sems[b, ci], 16)
                parts16.append(fold_to_64(xin[b][:, o * D : (o + cg) * D], cg))
            part = scratch(D, f32)
            last = nc.vector.tensor_tensor(
                part[:], parts16[0][:], parts16[1][:], op=add
            )
            for extra in parts16[2:]:
                last = nc.vector.tensor_tensor(part[:], part[:], extra[:], op=add)
            last.then_inc(s_dve, 1)

            nc.tensor.wait_ge(s_dve, b + 1)
            nc.tensor.matmul(
                tots[b][:], ones[:], part[:], start=True, stop=True,
            ).then_inc(s_pe, 1)

            nc.vector.wait_ge(s_pe, b + 1)
            nc.vector.tensor_copy(
                stage[:, b * D : (b + 1) * D], tots[b][:]
            ).then_inc(s_copy, 1)

        nc.sync.wait_ge(s_copy, B_LOCAL)
        nc.sync.dma_start(
            out=y.rearrange("b d -> (b d)").unsqueeze(0), in_=stage[:]
        ).then_inc(s_store, 16)
        nc.sync.wait_ge(s_store, 16)
        # Reset sems so a re-execution of the loaded NEFF starts clean.
        for s in [*load_sems.values(), s_dve, s_pe, s_copy, s_store]:
            nc.sync.sem_clear(s)

    return nc


def _get_nc():
    # _build_nc_raw (above) trims ~1 us of Tile preamble but its plain
    # .then_inc SWDGE completion sems release at descriptor emission, not
    # data landing -> races.  Use the proven TileContext build.
    if "nc" not in _STATE:
        _STATE["nc"] = _build_nc()
    return _STATE["nc"]


def kernel(node_j, edge_ij=None):
    global LAST_RESULT
    install_axon_ntff_hook_shim()
    from concourse.bass_utils import run_bass_kernel_spmd

    node_j = np.asarray(node_j)
    assert node_j.shape == (B, SIZE, D), node_j.shape
    x16 = np.ascontiguousarray(node_j, dtype=np.float16)

    nc = _get_nc()
    in_maps = [
        {"x": x16[i * B_LOCAL:(i + 1) * B_LOCAL]} for i in range(N_CORES)
    ]
    res = run_bass_kernel_spmd(nc, in_maps, core_ids=list(range(N_CORES)))
    LAST_RESULT = res
    sums = np.concatenate([r["y"] for r in res.results], axis=0)  # [16, 64]
    out = np.empty((B, SIZE, D), dtype=np.float32)
    np.copyto(out, sums[:, None, :])
    return out

